# revision 7
# baseline (speedup 1.0000x reference)
"""Trainium2 Bass kernel for nn_ClusteringLSTM (moe_routing).

Strategy (8 NeuronCores, SPMD):
  - The 2-layer LSTM (batch=1, T=2048, H=512) is inherently sequential, so it
    is REPLICATED on all 8 cores (no per-step cross-core sync possible under
    the ~20us collective latency floor). Each core therefore has the full
    lstm_out locally.
  - The 8 cluster heads (Wc[c]: [512, 20000]) are expert-parallel: core c owns
    head c, gathers exactly the rows with clusters==c (host-computed routing
    indices -> indirect DMA), computes logits, logsumexp, per-row NLL and an
    exact top-16 (hardware max8/max_index/match_replace), i.e. 1/8 of the
    naive head work per core.
  - Host: embedding-gather sharding prep, routing permutation, and final
    scatter/assembly (loss = sum of per-cluster mean NLL).

LSTM step mapping per layer: gates[2048] = W_hh.T-streamed matvec (M=1,
h stationary as 4 [128,1] K-tiles) + per-step row of the precomputed
X = x @ W_ih.T + b (chunk-batched matmuls, 64 steps at a time), accumulated in
PSUM; sigmoid/tanh on ACT (single table set); c/h elementwise on DVE; h
re-transposed via PE for the next step's lhsT.
"""
import numpy as np

B, E, H, V, C = 2048, 256, 512, 20000, 8
G = 4 * H            # gates per layer
V2 = 20480           # vocab padded to 40*512
CHUNK = 64
NCHUNKS = B // CHUNK  # 32
NCAP = 384            # padded per-core routed rows (3 tiles of 128)
HALF = V2 // 2        # 10240 (<= 16384 for max8)

_CACHE = {}


def _apply_bass_patches(tile, bass_rust, ScopedClock):
    """This walrus build accepts at most ONE sync-wait per instruction; Tile
    can attach several. Patch the tail drain; split_multi_waits handles the
    rest post-scheduling."""

    def _patched_drain_and_barrier(self, tick_clock, wait_clock):
        drain_inst = self.nc.sync.drain()
        wait_clock.add_sem_waits(
            drain_inst.ins, ScopedClock({None: tick_clock.global_clock})
        )
        si = drain_inst.ins.sync_info
        waits = list(si.on_wait) if si is not None else []
        if len(waits) > 1:
            drain_inst.ins.sync_info = bass_rust.SyncInfo(
                on_wait=[waits[0]], on_update=list(si.on_update)
            )
            for w in waits[1:]:
                nop = self.nc.sync.nop()
                nop.ins.sync_info = bass_rust.SyncInfo(on_wait=[w], on_update=[])
        self.nc.all_engine_barrier()
        assert self.sems is not None
        popped = self.nc._tile_sem_poison_stack.pop()
        assert popped is self._sem_poison
        self.nc.clear_and_free_semaphores(list(self.sems.allocated().values()))
        self.nc.all_engine_barrier()

    tile.TileContext._drain_and_barrier = _patched_drain_and_barrier


def _split_multi_waits(nc, bass_rust):
    n_split = 0
    main_bb = nc.cur_bb.bb if nc.cur_bb is not None else None
    for func in nc.m.functions:
        for block in func.blocks:
            il = block.instructions
            multi = {
                inst.name
                for inst in il
                if inst.sync_info is not None and len(inst.sync_info.on_wait) > 1
            }
            if not multi:
                continue
            new_list = []
            for inst in il:
                if inst.name in multi:
                    si = inst.sync_info
                    waits = list(si.on_wait)
                    for w in waits[:-1]:
                        nop = nc.engines[inst.engine].nop()
                        if main_bb is not None:
                            cur_il = main_bb.instructions
                            for k in range(len(cur_il) - 1, -1, -1):
                                if cur_il[k].name == nop.ins.name:
                                    cur_il.pop(k)
                                    break
                        nop.ins.sync_info = bass_rust.SyncInfo(
                            on_wait=[w], on_update=[]
                        )
                        new_list.append(nop.ins)
                        n_split += 1
                    inst.sync_info = bass_rust.SyncInfo(
                        on_wait=[waits[-1]], on_update=list(si.on_update)
                    )
                new_list.append(inst)
            il[:] = new_list
    return n_split


def _build_program():
    import concourse.bass as bass
    import concourse.tile as tile
    import bass_rust
    from concourse import mybir
    from concourse.bass import ds
    from concourse.masks import make_identity
    from concourse.vector_clock import ScopedClock

    _apply_bass_patches(tile, bass_rust, ScopedClock)

    f32 = mybir.dt.float32
    AF = mybir.ActivationFunctionType
    OP = mybir.AluOpType
    AX = mybir.AxisListType

    nc = bass.Bass("TRN2", target_bir_lowering=False, debug=False, num_devices=1)

    # ---- DRAM I/O ----
    d_xT = nc.dram_tensor("xT", [H, B], f32, kind="ExternalInput")
    d_wih0T = nc.dram_tensor("wih0T", [H, G], f32, kind="ExternalInput")
    d_whh0T = nc.dram_tensor("whh0T", [H, G], f32, kind="ExternalInput")
    d_wih1T = nc.dram_tensor("wih1T", [H, G], f32, kind="ExternalInput")
    d_whh1T = nc.dram_tensor("whh1T", [H, G], f32, kind="ExternalInput")
    d_b0 = nc.dram_tensor("b0", [1, G], f32, kind="ExternalInput")
    d_b1 = nc.dram_tensor("b1", [1, G], f32, kind="ExternalInput")
    d_hcinit = nc.dram_tensor("hcinit", [4, H], f32, kind="ExternalInput")
    d_ridx = nc.dram_tensor("ridx", [NCAP, 1], mybir.dt.int32, kind="ExternalInput")
    d_wt = nc.dram_tensor("wt", [NCAP, H], f32, kind="ExternalInput")
    d_bct = nc.dram_tensor("bct", [NCAP, 1], f32, kind="ExternalInput")
    d_wc = nc.dram_tensor("wc", [H, V2], f32, kind="ExternalInput")
    d_bcv = nc.dram_tensor("bcv", [1, V2], f32, kind="ExternalInput")
    d_iota = nc.dram_tensor("iota32", [128, 32], f32, kind="ExternalInput")

    d_lstm = nc.dram_tensor("lstm_out", [B, H], f32)  # internal

    d_nll = nc.dram_tensor("nll_out", [NCAP, 1], f32, kind="ExternalOutput")
    d_pred = nc.dram_tensor("pred_out", [NCAP, 16], f32, kind="ExternalOutput")
    d_state = nc.dram_tensor("state_out", [4, H], f32, kind="ExternalOutput")

    with tile.TileContext(nc) as tc:
        with tc.tile_pool(name="persist", bufs=1) as pers, \
             tc.tile_pool(name="ptr", bufs=2, space="PSUM") as ptr:

            ident = pers.tile([128, 128], f32)
            make_identity(nc, ident[:])
            ones1 = pers.tile([1, 1], f32)
            nc.vector.memset(ones1[:], 1.0)
            ones64 = pers.tile([1, CHUNK], f32)
            nc.vector.memset(ones64[:], 1.0)
            ones128 = pers.tile([1, 128], f32)
            nc.vector.memset(ones128[:], 1.0)

            # resident W_hh (streamed through PE every step)
            whh0 = pers.tile([128, 4 * G], f32, tag="whh0")
            whh1 = pers.tile([128, 4 * G], f32, tag="whh1")
            whh = [whh0, whh1]
            for l, src in enumerate([d_whh0T, d_whh1T]):
                for j in range(4):
                    nc.sync.dma_start(whh[l][:, j * G:(j + 1) * G],
                                      src[j * 128:(j + 1) * 128, :])
            brow0 = pers.tile([1, G], f32, tag="b0r")
            brow1 = pers.tile([1, G], f32, tag="b1r")
            brow = [brow0, brow1]
            nc.sync.dma_start(brow[0][:], d_b0[:])
            nc.sync.dma_start(brow[1][:], d_b1[:])

            # c state, one base-0 tile per layer (engines need partition base 0)
            cst0 = pers.tile([1, H], f32)
            nc.sync.dma_start(cst0[:], d_hcinit[2:3, :])
            cst1 = pers.tile([1, H], f32)
            nc.sync.dma_start(cst1[:], d_hcinit[3:4, :])
            cst = [cst0, cst1]
            # initial h, transposed: cols j = h0 K-tile j, cols 4+j = h1
            hT_init = pers.tile([128, 8], f32)
            hc_h = pers.tile([2, H], f32)
            nc.sync.dma_start(hc_h[:], d_hcinit[0:2, :])
            for j in range(4):
                ps = ptr.tile([128, 2], f32, tag="tr")
                nc.tensor.transpose(ps[:], hc_h[0:2, j * 128:(j + 1) * 128],
                                    ident[0:2, 0:2])
                nc.vector.tensor_copy(hT_init[:, j:j + 1], ps[:, 0:1])
                nc.vector.tensor_copy(hT_init[:, 4 + j:5 + j], ps[:, 1:2])

            # h1 lhsT (updated every L1 step)
            h1T = pers.tile([128, 4], f32)
            nc.vector.tensor_copy(h1T[:], hT_init[:, 4:8])

            # y0T parity buffers: transposed layer-0 outputs for one chunk
            y0T_e = pers.tile([128, 4 * CHUNK], f32, tag="y0Te")
            y0T_o = pers.tile([128, 4 * CHUNK], f32, tag="y0To")
            y0T = [y0T_e, y0T_o]

            with tc.tile_pool(name="xa", bufs=2) as xpool, \
                 tc.tile_pool(name="wr", bufs=6) as wrpool, \
                 tc.tile_pool(name="xbuf", bufs=2) as xbufp, \
                 tc.tile_pool(name="x1buf", bufs=2) as x1bufp, \
                 tc.tile_pool(name="gsb", bufs=4) as gsb, \
                 tc.tile_pool(name="hrow", bufs=3) as hrowp, \
                 tc.tile_pool(name="lacc", bufs=2) as laccp, \
                 tc.tile_pool(name="pg", bufs=4, space="PSUM") as pg, \
                 tc.tile_pool(name="pchunk", bufs=2, space="PSUM") as pchunk:

                def emit_x0chunk(sstart):
                    """X0 rows for 64 steps: x[c*64:(c+1)*64] @ W_ih0.T + b0."""
                    xt = xpool.tile([128, 4 * CHUNK], f32, tag="xt")
                    for j in range(4):
                        nc.sync.dma_start(
                            xt[:, j * CHUNK:(j + 1) * CHUNK],
                            d_xT[j * 128:(j + 1) * 128, ds(sstart, CHUNK)])
                    xb = xbufp.tile([CHUNK, G], f32, tag="x0")
                    for n in range(4):
                        ps = pchunk.tile([CHUNK, 512], f32, tag="pc")
                        for j in range(4):
                            w = wrpool.tile([128, 512], f32, tag="wrt")
                            nc.sync.dma_start(
                                w[:], d_wih0T[j * 128:(j + 1) * 128,
                                              n * 512:(n + 1) * 512])
                            nc.tensor.matmul(ps[:], lhsT=xt[:, j * CHUNK:(j + 1) * CHUNK],
                                             rhs=w[:], start=(j == 0), stop=False)
                        nc.tensor.matmul(ps[:], lhsT=ones64[:],
                                         rhs=brow[0][:, n * 512:(n + 1) * 512],
                                         start=False, stop=True)
                        nc.vector.tensor_copy(xb[:, n * 512:(n + 1) * 512], ps[:])
                    return xb

                def emit_x1chunk(parity):
                    """X1 rows from y0T[parity]: y0_chunk @ W_ih1.T + b1."""
                    src = y0T[parity]
                    xb = x1bufp.tile([CHUNK, G], f32, tag="x1")
                    for n in range(4):
                        ps = pchunk.tile([CHUNK, 512], f32, tag="pc")
                        for j in range(4):
                            w = wrpool.tile([128, 512], f32, tag="wrt")
                            nc.sync.dma_start(
                                w[:], d_wih1T[j * 128:(j + 1) * 128,
                                              n * 512:(n + 1) * 512])
                            nc.tensor.matmul(ps[:], lhsT=src[:, j * CHUNK:(j + 1) * CHUNK],
                                             rhs=w[:], start=(j == 0), stop=False)
                        nc.tensor.matmul(ps[:], lhsT=ones64[:],
                                         rhs=brow[1][:, n * 512:(n + 1) * 512],
                                         start=False, stop=True)
                        nc.vector.tensor_copy(xb[:, n * 512:(n + 1) * 512], ps[:])
                    return xb

                GATE_FUNC = [AF.Sigmoid, AF.Sigmoid, AF.Tanh, AF.Sigmoid]

                def emit_gates(l, lhsT_cols, xb, u, h_out):
                    """One LSTM cell step for layer l. lhsT_cols(j) -> [128,1]
                    h columns; xb = X buffer (row u); h_out [1, H] target."""
                    gates = []
                    for n in range(4):
                        g = pg.tile([1, 512], f32, tag="g")
                        for j in range(4):
                            nc.tensor.matmul(
                                g[:], lhsT=lhsT_cols(j),
                                rhs=whh[l][:, j * G + n * 512: j * G + (n + 1) * 512],
                                start=(j == 0), stop=False)
                        nc.tensor.matmul(g[:], lhsT=ident[0:CHUNK, u:u + 1],
                                         rhs=xb[:, n * 512:(n + 1) * 512],
                                         start=False, stop=True)
                        gs = gsb.tile([1, 512], f32, tag="gs")
                        nc.scalar.activation(gs[:], g[:], GATE_FUNC[n])
                        gates.append(gs)
                    gi, gf, gg, go = gates
                    t1 = gsb.tile([1, H], f32, tag="t1")
                    nc.vector.tensor_tensor(t1[:], gi[:], gg[:], op=OP.mult)
                    t2 = gsb.tile([1, H], f32, tag="t2")
                    nc.vector.tensor_tensor(t2[:], gf[:], cst[l][:], op=OP.mult)
                    nc.vector.tensor_tensor(cst[l][:], t1[:], t2[:], op=OP.add)
                    tc_ = gsb.tile([1, H], f32, tag="tc")
                    nc.scalar.activation(tc_[:], cst[l][:], AF.Tanh)
                    nc.vector.tensor_tensor(h_out[:], go[:], tc_[:], op=OP.mult)

                def emit_pair(u, l0, l1):
                    """Emit one slot: optionally L0 step (chunk ca, local u)
                    and L1 step (chunk ca-1, local u). l0/l1 are dicts or None."""
                    hrow0 = hrow1 = None
                    if l0 is not None:
                        hrow0 = hrowp.tile([1, H], f32, tag="hr0")
                        emit_gates(0, l0["lhsT"], l0["xb"], u, hrow0[:])
                        for j in range(4):
                            ps = ptr.tile([128, 2], f32, tag="tr")
                            nc.tensor.transpose(ps[:, 0:1],
                                                hrow0[0:1, j * 128:(j + 1) * 128],
                                                ident[0:1, 0:1])
                            nc.vector.tensor_copy(
                                l0["y0T"][:, j * CHUNK + u: j * CHUNK + u + 1],
                                ps[:, 0:1])
                    if l1 is not None:
                        hrow1 = hrowp.tile([1, H], f32, tag="hr1")
                        emit_gates(1, lambda j: h1T[:, j:j + 1], l1["xb"], u,
                                   hrow1[:])
                        nc.sync.dma_start(l1["acc"][u:u + 1, :], hrow1[:])
                        for j in range(4):
                            ps = ptr.tile([128, 2], f32, tag="tr")
                            nc.tensor.transpose(ps[:, 0:1],
                                                hrow1[0:1, j * 128:(j + 1) * 128],
                                                ident[0:1, 0:1])
                            nc.vector.tensor_copy(h1T[:, j:j + 1], ps[:, 0:1])
                    return hrow0 if hrow0 is not None else hrow1

                def l0_cols(y0T_cur, y0T_prev, u):
                    if u == 0:
                        return lambda j: y0T_prev[:, j * CHUNK + CHUNK - 1:
                                                  j * CHUNK + CHUNK]
                    return lambda j: y0T_cur[:, j * CHUNK + u - 1: j * CHUNK + u]

                # ---- prologue: L0 chunk 0 ----
                xb0 = emit_x0chunk(0)
                hT_init_cols = lambda j: hT_init[:, j:j + 1]
                for u in range(CHUNK):
                    cols = hT_init_cols if u == 0 else \
                        (lambda uu: (lambda j: y0T[0][:, j * CHUNK + uu - 1:
                                                      j * CHUNK + uu]))(u)
                    emit_pair(u, {"lhsT": cols, "xb": xb0, "y0T": y0T[0]}, None)

                # ---- main loop: bodies of 2 chunks (a=2j+1, b=2j+2) ----
                with tc.For_i(0, 15 * 2 * CHUNK, 2 * CHUNK) as jv:
                    xb_a = emit_x0chunk(jv + CHUNK)
                    xb_b = emit_x0chunk(jv + 2 * CHUNK)
                    x1_a = emit_x1chunk(0)     # chunk 2j (even parity)
                    acc_a = laccp.tile([CHUNK, H], f32, tag="lacc")
                    for u in range(CHUNK):
                        emit_pair(
                            u,
                            {"lhsT": l0_cols(y0T[1], y0T[0], u), "xb": xb_a,
                             "y0T": y0T[1]},
                            {"xb": x1_a, "acc": acc_a},
                        )
                    nc.sync.dma_start(d_lstm[ds(jv, CHUNK), :], acc_a[:])
                    x1_b = emit_x1chunk(1)     # chunk 2j+1 (odd parity)
                    acc_b = laccp.tile([CHUNK, H], f32, tag="lacc")
                    for u in range(CHUNK):
                        emit_pair(
                            u,
                            {"lhsT": l0_cols(y0T[0], y0T[1], u), "xb": xb_b,
                             "y0T": y0T[0]},
                            {"xb": x1_b, "acc": acc_b},
                        )
                    nc.sync.dma_start(d_lstm[ds(jv + CHUNK, CHUNK), :], acc_b[:])

                # ---- epilogue: L0 chunk 31 ; L1 chunks 30, 31 ----
                xb31 = emit_x0chunk(31 * CHUNK)
                last_h0 = None
                for u in range(CHUNK):
                    last_h0 = emit_pair(
                        u, {"lhsT": l0_cols(y0T[1], y0T[0], u), "xb": xb31,
                            "y0T": y0T[1]}, None)
                x1_30 = emit_x1chunk(0)
                acc30 = laccp.tile([CHUNK, H], f32, tag="lacc")
                for u in range(CHUNK):
                    emit_pair(u, None, {"xb": x1_30, "acc": acc30})
                nc.sync.dma_start(d_lstm[30 * CHUNK:31 * CHUNK, :], acc30[:])
                x1_31 = emit_x1chunk(1)
                acc31 = laccp.tile([CHUNK, H], f32, tag="lacc")
                last_h1 = None
                for u in range(CHUNK):
                    last_h1 = emit_pair(u, None, {"xb": x1_31, "acc": acc31})
                nc.sync.dma_start(d_lstm[31 * CHUNK:32 * CHUNK, :], acc31[:])

                # final state rows: h0, h1, c0, c1
                nc.sync.dma_start(d_state[0:1, :], last_h0[:])
                nc.sync.dma_start(d_state[1:2, :], last_h1[:])
                nc.sync.dma_start(d_state[2:3, :], cst0[:])
                nc.sync.dma_start(d_state[3:4, :], cst1[:])

            # ================= heads =================
            with tc.tile_pool(name="hd", bufs=1) as hd, \
                 tc.tile_pool(name="hdw", bufs=4) as hdw, \
                 tc.tile_pool(name="hsc", bufs=1) as hsc, \
                 tc.tile_pool(name="ph", bufs=4, space="PSUM") as ph, \
                 tc.tile_pool(name="ptr2", bufs=2, space="PSUM") as ptr2:

                iota = pers.tile([128, 32], f32)
                nc.sync.dma_start(iota[:], d_iota[:])

                logits = hd.tile([128, V2], f32, tag="logits")

                for mt in range(3):
                    # gather this tile's 128 routed lstm_out rows
                    ridx_sb = hsc.tile([128, 1], mybir.dt.int32, tag="ridx")
                    nc.sync.dma_start(ridx_sb[:], d_ridx[mt * 128:(mt + 1) * 128, :])
                    R = hsc.tile([128, H], f32, tag="R")
                    nc.gpsimd.indirect_dma_start(
                        out=R[:], out_offset=None, in_=d_lstm[:],
                        in_offset=bass.IndirectOffsetOnAxis(ap=ridx_sb[:, :1], axis=0),
                    )
                    # RT: [512, 128] as 4 [128,128] K-tiles
                    RT = hsc.tile([128, 4 * 128], f32, tag="RT")
                    for j in range(4):
                        ps = ptr2.tile([128, 128], f32, tag="tr2")
                        nc.tensor.transpose(ps[:], R[:, j * 128:(j + 1) * 128],
                                            ident[:])
                        nc.vector.tensor_copy(RT[:, j * 128:(j + 1) * 128], ps[:])

                    # logits = R @ Wc + bc
                    for n in range(V2 // 512):
                        ps = ph.tile([128, 512], f32, tag="ph")
                        for j in range(4):
                            w = hdw.tile([128, 512], f32, tag="hw")
                            nc.sync.dma_start(
                                w[:], d_wc[j * 128:(j + 1) * 128,
                                           n * 512:(n + 1) * 512])
                            nc.tensor.matmul(ps[:], lhsT=RT[:, j * 128:(j + 1) * 128],
                                             rhs=w[:], start=(j == 0), stop=False)
                        bv = hdw.tile([1, 512], f32, tag="hb")
                        nc.sync.dma_start(bv[:], d_bcv[:, n * 512:(n + 1) * 512])
                        nc.tensor.matmul(ps[:], lhsT=ones128[:], rhs=bv[:],
                                         start=False, stop=True)
                        nc.vector.tensor_copy(logits[:, n * 512:(n + 1) * 512], ps[:])

                    # row max via max8 on both halves
                    m8a = hsc.tile([128, 8], f32, tag="m8a")
                    m8b = hsc.tile([128, 8], f32, tag="m8b")
                    nc.vector.max(out=m8a[:], in_=logits[:, :HALF])
                    nc.vector.max(out=m8b[:], in_=logits[:, HALF:])
                    negmax = hsc.tile([128, 1], f32, tag="negmax")
                    nc.vector.tensor_tensor(negmax[:], m8a[:, 0:1], m8b[:, 0:1],
                                            op=OP.max)
                    nc.vector.tensor_scalar_mul(negmax[:], negmax[:], -1.0)

                    # logsumexp: exp(x - max) chunk-wise with accumulation
                    esums = hsc.tile([128, V2 // 512], f32, tag="esums")
                    escr = hsc.tile([128, 512], f32, tag="escr")
                    for n in range(V2 // 512):
                        nc.scalar.activation(escr[:], logits[:, n * 512:(n + 1) * 512],
                                             AF.Exp, bias=negmax[:],
                                             accum_out=esums[:, n:n + 1])
                    lse = hsc.tile([128, 1], f32, tag="lse")
                    nc.vector.tensor_reduce(lse[:], esums[:], axis=AX.X, op=OP.add)
                    nc.scalar.activation(lse[:], lse[:], AF.Ln)
                    nc.vector.tensor_scalar(lse[:], lse[:], negmax[:], None,
                                            op0=OP.subtract)

                    # nll = lse - (sum(R*wt) + bct)
                    wt_sb = hsc.tile([128, H], f32, tag="wt")
                    nc.sync.dma_start(wt_sb[:], d_wt[mt * 128:(mt + 1) * 128, :])
                    dot = hsc.tile([128, 1], f32, tag="dot")
                    dscr = hsc.tile([128, H], f32, tag="dscr")
                    nc.vector.scalar_tensor_tensor(
                        dscr[:], R[:], 1.0, wt_sb[:], op0=OP.mult, op1=OP.mult,
                        accum_out=dot[:])
                    bct_sb = hsc.tile([128, 1], f32, tag="bct")
                    nc.sync.dma_start(bct_sb[:], d_bct[mt * 128:(mt + 1) * 128, :])
                    nll = hsc.tile([128, 1], f32, tag="nll")
                    nc.vector.tensor_tensor(nll[:], dot[:], bct_sb[:], op=OP.add)
                    nc.vector.tensor_tensor(nll[:], lse[:], nll[:], op=OP.subtract)
                    nc.sync.dma_start(d_nll[mt * 128:(mt + 1) * 128, :], nll[:])

                    # ---- exact top-16 ----
                    # per half: top-8 (+idx), zap, next top-8 (+idx)
                    cand_v = hsc.tile([128, 32], f32, tag="cand_v")
                    cand_i = hsc.tile([128, 32], f32, tag="cand_i")
                    iu = hsc.tile([128, 8], mybir.dt.uint32, tag="iu")
                    for hix, (lo, hi) in enumerate([(0, HALF), (HALF, V2)]):
                        half = logits[:, lo:hi]
                        for r in range(2):
                            co = hix * 16 + r * 8
                            m8 = hsc.tile([128, 8], f32, tag="m8")
                            nc.vector.max(out=m8[:], in_=half)
                            nc.vector.max_index(out=iu[:], in_max=m8[:], in_values=half)
                            nc.vector.tensor_copy(cand_v[:, co:co + 8], m8[:])
                            nc.vector.tensor_copy(cand_i[:, co:co + 8], iu[:])
                            if hix == 1:
                                nc.vector.tensor_scalar_add(
                                    cand_i[:, co:co + 8], cand_i[:, co:co + 8],
                                    float(HALF))
                            if r == 0:
                                nc.vector.match_replace(out=half, in_to_replace=m8[:],
                                                        in_values=half,
                                                        imm_value=-1e30)
                    # top-16 of the 32 candidates
                    pred = hsc.tile([128, 16], f32, tag="pred")
                    pos = hsc.tile([128, 16], f32, tag="pos")
                    cv_copy = hsc.tile([128, 32], f32, tag="cvc")
                    nc.vector.tensor_copy(cv_copy[:], cand_v[:])
                    m16 = hsc.tile([128, 8], f32, tag="m16")
                    p8 = hsc.tile([128, 8], mybir.dt.uint32, tag="p8")
                    for r in range(2):
                        nc.vector.max(out=m16[:], in_=cv_copy[:])
                        nc.vector.max_index(out=p8[:], in_max=m16[:], in_values=cand_v[:])
                        nc.vector.tensor_copy(pos[:, r * 8:(r + 1) * 8], p8[:])
                        if r == 0:
                            nc.vector.match_replace(out=cv_copy[:], in_to_replace=m16[:],
                                                    in_values=cv_copy[:],
                                                    imm_value=-1e30)
                    # map candidate positions -> original vocab indices
                    eq = hsc.tile([128, 32], f32, tag="eq")
                    for kk in range(16):
                        nc.vector.tensor_scalar(eq[:], iota[:], pos[:, kk:kk + 1],
                                                None, op0=OP.is_equal)
                        nc.vector.scalar_tensor_tensor(
                            eq[:], eq[:], 1.0, cand_i[:], op0=OP.mult, op1=OP.mult,
                            accum_out=pred[:, kk:kk + 1])
                    nc.sync.dma_start(d_pred[mt * 128:(mt + 1) * 128, :], pred[:])

    _split_multi_waits(nc, bass_rust)
    return nc


def _get_program():
    if "nc" not in _CACHE:
        _CACHE["nc"] = _build_program()
    return _CACHE["nc"]


def kernel(pc, delta, clusters, target, h0, c0, pc_embed_w, delta_embed_w,
           W_ih0, W_hh0, b_ih0, b_hh0, W_ih1, W_hh1, b_ih1, b_hh1, Wc, bc):
    from concourse.bass_utils import run_bass_kernel_spmd

    pc = np.asarray(pc).astype(np.int64)
    delta = np.asarray(delta).astype(np.int64)
    clusters_np = np.asarray(clusters).astype(np.int64)
    target_np = np.asarray(target).astype(np.int64)
    f = lambda a: np.ascontiguousarray(np.asarray(a), dtype=np.float32)
    h0 = f(h0); c0 = f(c0)
    Wc = f(Wc); bc = f(bc)

    # host prep: embedding gather + transposes (pure data movement / sharding)
    x = np.concatenate([f(pc_embed_w)[pc], f(delta_embed_w)[delta]], axis=-1)
    xT = np.ascontiguousarray(x.T)                     # [512, 2048]
    wih0T = np.ascontiguousarray(f(W_ih0).T)           # [512, 2048]
    whh0T = np.ascontiguousarray(f(W_hh0).T)
    wih1T = np.ascontiguousarray(f(W_ih1).T)
    whh1T = np.ascontiguousarray(f(W_hh1).T)
    b0 = (f(b_ih0) + f(b_hh0)).reshape(1, G)
    b1 = (f(b_ih1) + f(b_hh1)).reshape(1, G)
    hcinit = np.stack([h0[0], h0[1], c0[0], c0[1]]).astype(np.float32)
    iota32 = np.tile(np.arange(32, dtype=np.float32), (128, 1))

    common = dict(xT=xT, wih0T=wih0T, whh0T=whh0T, wih1T=wih1T, whh1T=whh1T,
                  b0=b0, b1=b1, hcinit=hcinit, iota32=iota32)

    rows_by_c = [np.where(clusters_np == c)[0] for c in range(C)]
    counts = [len(r) for r in rows_by_c]
    assert max(counts) <= NCAP, f"cluster count {max(counts)} exceeds NCAP={NCAP}"

    in_maps = []
    for c in range(C):
        rows = rows_by_c[c]
        ridx = np.zeros(NCAP, dtype=np.int32)
        ridx[:len(rows)] = rows
        tr = target_np[ridx.astype(np.int64)]          # padded rows use row 0's
        WcT = np.ascontiguousarray(Wc[c].T)            # [20000, 512]
        wt = WcT[tr]                                   # [NCAP, 512]
        bct = bc[c][tr].reshape(NCAP, 1)
        wc_pad = np.zeros((H, V2), dtype=np.float32)
        wc_pad[:, :V] = Wc[c]
        bcv = np.full((1, V2), -1e30, dtype=np.float32)
        bcv[0, :V] = bc[c]
        m = dict(common)
        m.update(ridx=ridx.reshape(NCAP, 1), wt=np.ascontiguousarray(wt),
                 bct=np.ascontiguousarray(bct), wc=wc_pad, bcv=bcv)
        in_maps.append(m)

    nc = _get_program()
    import os
    trace = bool(int(os.environ.get("KERNEL_TRACE", "0")))
    res = run_bass_kernel_spmd(nc, in_maps, core_ids=list(range(C)), trace=trace)
    _CACHE["exec_time_ns"] = res.exec_time_ns
    _CACHE["results"] = res

    # ---- assemble outputs ----
    preds = np.zeros((B, 10), dtype=np.int32)
    loss = np.float32(0.0)
    for c in range(C):
        cnt = counts[c]
        if cnt == 0:
            continue
        rows = rows_by_c[c]
        nll = res.results[c]["nll_out"][:cnt, 0]
        loss = np.float32(loss + np.float32(nll.sum() / np.float32(cnt)))
        p = res.results[c]["pred_out"][:cnt, :10]
        preds[rows] = p.astype(np.int32)
    st = res.results[0]["state_out"]
    state = (np.stack([st[0], st[1]]).astype(np.float32),
             np.stack([st[2], st[3]]).astype(np.float32))
    return np.float32(loss), preds, state


# revision 8
# speedup vs baseline: 1.6278x; 1.6278x over previous
"""Trainium2 Bass kernel for nn_ClusteringLSTM (moe_routing).

Strategy (8 NeuronCores, SPMD):
  - The 2-layer LSTM (batch=1, T=2048, H=512) is inherently sequential, so it
    is REPLICATED on all 8 cores (no per-step cross-core sync possible under
    the ~20us collective latency floor). Each core therefore has the full
    lstm_out locally.
  - The 8 cluster heads (Wc[c]: [512, 20000]) are expert-parallel: core c owns
    head c, gathers exactly the rows with clusters==c (host-computed routing
    indices -> indirect DMA), computes logits, logsumexp, per-row NLL and an
    exact top-16 (hardware max8/max_index/match_replace), i.e. 1/8 of the
    naive head work per core.
  - Host: embedding-gather sharding prep, routing permutation, and final
    scatter/assembly (loss = sum of per-cluster mean NLL).

LSTM step mapping per layer: gates[2048] = W_hh.T-streamed matvec (M=1,
h stationary as 4 [128,1] K-tiles) + per-step row of the precomputed
X = x @ W_ih.T + b (chunk-batched matmuls, 64 steps at a time), accumulated in
PSUM; sigmoid/tanh on ACT (single table set); c/h elementwise on DVE; h
re-transposed via PE for the next step's lhsT.
"""
import numpy as np

B, E, H, V, C = 2048, 256, 512, 20000, 8
G = 4 * H            # gates per layer
V2 = 20480           # vocab padded to 40*512
CHUNK = 64
NCHUNKS = B // CHUNK  # 32
NCAP = 384            # padded per-core routed rows (3 tiles of 128)
HALF = V2 // 2        # 10240 (<= 16384 for max8)

_CACHE = {}


def _apply_bass_patches(tile, bass_rust, ScopedClock):
    """This walrus build accepts at most ONE sync-wait per instruction; Tile
    can attach several. Patch the tail drain; split_multi_waits handles the
    rest post-scheduling."""

    def _patched_drain_and_barrier(self, tick_clock, wait_clock):
        drain_inst = self.nc.sync.drain()
        wait_clock.add_sem_waits(
            drain_inst.ins, ScopedClock({None: tick_clock.global_clock})
        )
        si = drain_inst.ins.sync_info
        waits = list(si.on_wait) if si is not None else []
        if len(waits) > 1:
            drain_inst.ins.sync_info = bass_rust.SyncInfo(
                on_wait=[waits[0]], on_update=list(si.on_update)
            )
            for w in waits[1:]:
                nop = self.nc.sync.nop()
                nop.ins.sync_info = bass_rust.SyncInfo(on_wait=[w], on_update=[])
        self.nc.all_engine_barrier()
        assert self.sems is not None
        popped = self.nc._tile_sem_poison_stack.pop()
        assert popped is self._sem_poison
        self.nc.clear_and_free_semaphores(list(self.sems.allocated().values()))
        self.nc.all_engine_barrier()

    tile.TileContext._drain_and_barrier = _patched_drain_and_barrier


def _split_multi_waits(nc, bass_rust):
    n_split = 0
    main_bb = nc.cur_bb.bb if nc.cur_bb is not None else None
    for func in nc.m.functions:
        for block in func.blocks:
            il = block.instructions
            multi = {
                inst.name
                for inst in il
                if inst.sync_info is not None and len(inst.sync_info.on_wait) > 1
            }
            if not multi:
                continue
            new_list = []
            for inst in il:
                if inst.name in multi:
                    si = inst.sync_info
                    waits = list(si.on_wait)
                    for w in waits[:-1]:
                        nop = nc.engines[inst.engine].nop()
                        if main_bb is not None:
                            cur_il = main_bb.instructions
                            for k in range(len(cur_il) - 1, -1, -1):
                                if cur_il[k].name == nop.ins.name:
                                    cur_il.pop(k)
                                    break
                        nop.ins.sync_info = bass_rust.SyncInfo(
                            on_wait=[w], on_update=[]
                        )
                        new_list.append(nop.ins)
                        n_split += 1
                    inst.sync_info = bass_rust.SyncInfo(
                        on_wait=[waits[-1]], on_update=list(si.on_update)
                    )
                new_list.append(inst)
            il[:] = new_list
    return n_split


def _build_program():
    import concourse.bass as bass
    import concourse.tile as tile
    import bass_rust
    from concourse import mybir
    from concourse.bass import ds
    from concourse.masks import make_identity
    from concourse.vector_clock import ScopedClock

    _apply_bass_patches(tile, bass_rust, ScopedClock)

    f32 = mybir.dt.float32
    AF = mybir.ActivationFunctionType
    OP = mybir.AluOpType
    AX = mybir.AxisListType

    nc = bass.Bass("TRN2", target_bir_lowering=False, debug=False, num_devices=1)

    # ---- DRAM I/O ----
    d_xT = nc.dram_tensor("xT", [H, B], f32, kind="ExternalInput")
    d_wih0T = nc.dram_tensor("wih0T", [H, G], f32, kind="ExternalInput")
    d_whh0T = nc.dram_tensor("whh0T", [H, G], f32, kind="ExternalInput")
    d_wih1T = nc.dram_tensor("wih1T", [H, G], f32, kind="ExternalInput")
    d_whh1T = nc.dram_tensor("whh1T", [H, G], f32, kind="ExternalInput")
    d_b0 = nc.dram_tensor("b0", [1, G], f32, kind="ExternalInput")
    d_b1 = nc.dram_tensor("b1", [1, G], f32, kind="ExternalInput")
    d_hcinit = nc.dram_tensor("hcinit", [4, H], f32, kind="ExternalInput")
    d_ridx = nc.dram_tensor("ridx", [NCAP, 1], mybir.dt.int32, kind="ExternalInput")
    d_wt = nc.dram_tensor("wt", [NCAP, H], f32, kind="ExternalInput")
    d_bct = nc.dram_tensor("bct", [NCAP, 1], f32, kind="ExternalInput")
    d_wc = nc.dram_tensor("wc", [H, V2], f32, kind="ExternalInput")
    d_bcv = nc.dram_tensor("bcv", [1, V2], f32, kind="ExternalInput")
    d_iota = nc.dram_tensor("iota32", [128, 32], f32, kind="ExternalInput")

    d_lstm = nc.dram_tensor("lstm_out", [B, H], f32)  # internal

    d_nll = nc.dram_tensor("nll_out", [NCAP, 1], f32, kind="ExternalOutput")
    d_pred = nc.dram_tensor("pred_out", [NCAP, 16], f32, kind="ExternalOutput")
    d_state = nc.dram_tensor("state_out", [4, H], f32, kind="ExternalOutput")

    with tile.TileContext(nc) as tc:
        with tc.tile_pool(name="persist", bufs=1) as pers, \
             tc.tile_pool(name="ptr", bufs=2, space="PSUM") as ptr:

            ident = pers.tile([128, 128], f32)
            make_identity(nc, ident[:])
            ones1 = pers.tile([1, 1], f32)
            nc.vector.memset(ones1[:], 1.0)
            ones64 = pers.tile([1, CHUNK], f32)
            nc.vector.memset(ones64[:], 1.0)
            ones128 = pers.tile([1, 128], f32)
            nc.vector.memset(ones128[:], 1.0)
            identb = pers.tile([128, 128], mybir.dt.bfloat16)
            nc.vector.tensor_copy(identb[:], ident[:])

            # resident W_hh (streamed through PE every step)
            whh0 = pers.tile([128, 4 * G], f32, tag="whh0")
            whh1 = pers.tile([128, 4 * G], f32, tag="whh1")
            whh = [whh0, whh1]
            for l, src in enumerate([d_whh0T, d_whh1T]):
                for j in range(4):
                    nc.sync.dma_start(whh[l][:, j * G:(j + 1) * G],
                                      src[j * 128:(j + 1) * 128, :])
            brow0 = pers.tile([1, G], f32, tag="b0r")
            brow1 = pers.tile([1, G], f32, tag="b1r")
            brow = [brow0, brow1]
            nc.sync.dma_start(brow[0][:], d_b0[:])
            nc.sync.dma_start(brow[1][:], d_b1[:])

            # c state, one base-0 tile per layer (engines need partition base 0)
            cst0 = pers.tile([1, H], f32)
            nc.sync.dma_start(cst0[:], d_hcinit[2:3, :])
            cst1 = pers.tile([1, H], f32)
            nc.sync.dma_start(cst1[:], d_hcinit[3:4, :])
            cst = [cst0, cst1]
            # initial h, transposed: cols j = h0 K-tile j, cols 4+j = h1
            hT_init = pers.tile([128, 8], f32)
            hc_h = pers.tile([2, H], f32)
            nc.sync.dma_start(hc_h[:], d_hcinit[0:2, :])
            for j in range(4):
                ps = ptr.tile([128, 2], f32, tag="tr")
                nc.tensor.transpose(ps[:], hc_h[0:2, j * 128:(j + 1) * 128],
                                    ident[0:2, 0:2])
                nc.vector.tensor_copy(hT_init[:, j:j + 1], ps[:, 0:1])
                nc.vector.tensor_copy(hT_init[:, 4 + j:5 + j], ps[:, 1:2])

            # h1 lhsT (updated every L1 step)
            h1T = pers.tile([128, 4], f32)
            nc.vector.tensor_copy(h1T[:], hT_init[:, 4:8])

            # y0T parity buffers: transposed layer-0 outputs for one chunk
            y0T_e = pers.tile([128, 4 * CHUNK], f32, tag="y0Te")
            y0T_o = pers.tile([128, 4 * CHUNK], f32, tag="y0To")
            y0T = [y0T_e, y0T_o]

            with tc.tile_pool(name="xa", bufs=2) as xpool, \
                 tc.tile_pool(name="wr", bufs=6) as wrpool, \
                 tc.tile_pool(name="xbuf", bufs=2) as xbufp, \
                 tc.tile_pool(name="x1buf", bufs=2) as x1bufp, \
                 tc.tile_pool(name="gsb", bufs=4) as gsb, \
                 tc.tile_pool(name="hrow", bufs=3) as hrowp, \
                 tc.tile_pool(name="lacc", bufs=2) as laccp, \
                 tc.tile_pool(name="pg", bufs=4, space="PSUM") as pg, \
                 tc.tile_pool(name="pchunk", bufs=2, space="PSUM") as pchunk:

                def emit_x0chunk(sstart):
                    """X0 rows for 64 steps: x[c*64:(c+1)*64] @ W_ih0.T + b0."""
                    xt = xpool.tile([128, 4 * CHUNK], f32, tag="xt")
                    for j in range(4):
                        nc.sync.dma_start(
                            xt[:, j * CHUNK:(j + 1) * CHUNK],
                            d_xT[j * 128:(j + 1) * 128, ds(sstart, CHUNK)])
                    xh = xbufp.tile([CHUNK, G], mybir.dt.bfloat16, tag="x0h")
                    xl = xbufp.tile([CHUNK, G], mybir.dt.bfloat16, tag="x0l")
                    for n in range(4):
                        ps = pchunk.tile([CHUNK, 512], f32, tag="pc")
                        for j in range(4):
                            w = wrpool.tile([128, 512], f32, tag="wrt")
                            nc.sync.dma_start(
                                w[:], d_wih0T[j * 128:(j + 1) * 128,
                                              n * 512:(n + 1) * 512])
                            nc.tensor.matmul(ps[:], lhsT=xt[:, j * CHUNK:(j + 1) * CHUNK],
                                             rhs=w[:], start=(j == 0), stop=False)
                        nc.tensor.matmul(ps[:], lhsT=ones64[:],
                                         rhs=brow[0][:, n * 512:(n + 1) * 512],
                                         start=False, stop=True)
                        sl = slice(n * 512, (n + 1) * 512)
                        nc.vector.tensor_copy(xh[:, sl], ps[:])
                        nc.vector.tensor_tensor(xl[:, sl], ps[:], xh[:, sl],
                                                op=OP.subtract)
                    return (xh, xl)

                def emit_x1chunk(parity):
                    """X1 rows from y0T[parity]: y0_chunk @ W_ih1.T + b1."""
                    src = y0T[parity]
                    xh = x1bufp.tile([CHUNK, G], mybir.dt.bfloat16, tag="x1h")
                    xl = x1bufp.tile([CHUNK, G], mybir.dt.bfloat16, tag="x1l")
                    for n in range(4):
                        ps = pchunk.tile([CHUNK, 512], f32, tag="pc")
                        for j in range(4):
                            w = wrpool.tile([128, 512], f32, tag="wrt")
                            nc.sync.dma_start(
                                w[:], d_wih1T[j * 128:(j + 1) * 128,
                                              n * 512:(n + 1) * 512])
                            nc.tensor.matmul(ps[:], lhsT=src[:, j * CHUNK:(j + 1) * CHUNK],
                                             rhs=w[:], start=(j == 0), stop=False)
                        nc.tensor.matmul(ps[:], lhsT=ones64[:],
                                         rhs=brow[1][:, n * 512:(n + 1) * 512],
                                         start=False, stop=True)
                        sl = slice(n * 512, (n + 1) * 512)
                        nc.vector.tensor_copy(xh[:, sl], ps[:])
                        nc.vector.tensor_tensor(xl[:, sl], ps[:], xh[:, sl],
                                                op=OP.subtract)
                    return (xh, xl)

                GATE_FUNC = [AF.Sigmoid, AF.Sigmoid, AF.Tanh, AF.Sigmoid]

                def emit_gates(l, lhsT_cols, xb, u, h_out):
                    """One LSTM cell step for layer l. lhsT_cols(j) -> [128,1]
                    h columns; xb = X buffer (row u); h_out [1, H] target."""
                    gates = []
                    for n in range(4):
                        g = pg.tile([1, 512], f32, tag="g")
                        for j in range(4):
                            nc.tensor.matmul(
                                g[:], lhsT=lhsT_cols(j),
                                rhs=whh[l][:, j * G + n * 512: j * G + (n + 1) * 512],
                                start=(j == 0), stop=False)
                        xh, xl = xb
                        nc.tensor.matmul(g[:], lhsT=identb[0:CHUNK, u:u + 1],
                                         rhs=xh[:, n * 512:(n + 1) * 512],
                                         start=False, stop=False)
                        nc.tensor.matmul(g[:], lhsT=identb[0:CHUNK, u:u + 1],
                                         rhs=xl[:, n * 512:(n + 1) * 512],
                                         start=False, stop=True)
                        gs = gsb.tile([1, 512], f32, tag="gs")
                        nc.scalar.activation(gs[:], g[:], GATE_FUNC[n])
                        gates.append(gs)
                    gi, gf, gg, go = gates
                    t1 = gsb.tile([1, H], f32, tag="t1")
                    nc.vector.tensor_tensor(t1[:], gi[:], gg[:], op=OP.mult)
                    t2 = gsb.tile([1, H], f32, tag="t2")
                    nc.vector.tensor_tensor(t2[:], gf[:], cst[l][:], op=OP.mult)
                    nc.vector.tensor_tensor(cst[l][:], t1[:], t2[:], op=OP.add)
                    tc_ = gsb.tile([1, H], f32, tag="tc")
                    nc.scalar.activation(tc_[:], cst[l][:], AF.Tanh)
                    nc.vector.tensor_tensor(h_out[:], go[:], tc_[:], op=OP.mult)

                def emit_pair(u, l0, l1):
                    """Emit one slot: optionally L0 step (chunk ca, local u)
                    and L1 step (chunk ca-1, local u). l0/l1 are dicts or None."""
                    hrow0 = hrow1 = None
                    if l0 is not None:
                        hrow0 = hrowp.tile([1, H], f32, tag="hr0")
                        emit_gates(0, l0["lhsT"], l0["xb"], u, hrow0[:])
                        for j in range(4):
                            ps = ptr.tile([128, 2], f32, tag="tr")
                            nc.tensor.transpose(ps[:, 0:1],
                                                hrow0[0:1, j * 128:(j + 1) * 128],
                                                ident[0:1, 0:1])
                            nc.vector.tensor_copy(
                                l0["y0T"][:, j * CHUNK + u: j * CHUNK + u + 1],
                                ps[:, 0:1])
                    if l1 is not None:
                        hrow1 = hrowp.tile([1, H], f32, tag="hr1")
                        emit_gates(1, lambda j: h1T[:, j:j + 1], l1["xb"], u,
                                   hrow1[:])
                        nc.sync.dma_start(l1["acc"][u:u + 1, :], hrow1[:])
                        for j in range(4):
                            ps = ptr.tile([128, 2], f32, tag="tr")
                            nc.tensor.transpose(ps[:, 0:1],
                                                hrow1[0:1, j * 128:(j + 1) * 128],
                                                ident[0:1, 0:1])
                            nc.vector.tensor_copy(h1T[:, j:j + 1], ps[:, 0:1])
                    return hrow0 if hrow0 is not None else hrow1

                def l0_cols(y0T_cur, y0T_prev, u):
                    if u == 0:
                        return lambda j: y0T_prev[:, j * CHUNK + CHUNK - 1:
                                                  j * CHUNK + CHUNK]
                    return lambda j: y0T_cur[:, j * CHUNK + u - 1: j * CHUNK + u]

                # ---- prologue: L0 chunk 0 ----
                xb0 = emit_x0chunk(0)
                hT_init_cols = lambda j: hT_init[:, j:j + 1]
                for u in range(CHUNK):
                    cols = hT_init_cols if u == 0 else \
                        (lambda uu: (lambda j: y0T[0][:, j * CHUNK + uu - 1:
                                                      j * CHUNK + uu]))(u)
                    emit_pair(u, {"lhsT": cols, "xb": xb0, "y0T": y0T[0]}, None)

                # ---- main loop: bodies of 2 chunks (a=2j+1, b=2j+2) ----
                with tc.For_i(0, 15 * 2 * CHUNK, 2 * CHUNK) as jv:
                    xb_a = emit_x0chunk(jv + CHUNK)
                    xb_b = emit_x0chunk(jv + 2 * CHUNK)
                    x1_a = emit_x1chunk(0)     # chunk 2j (even parity)
                    acc_a = laccp.tile([CHUNK, H], f32, tag="lacc")
                    for u in range(CHUNK):
                        emit_pair(
                            u,
                            {"lhsT": l0_cols(y0T[1], y0T[0], u), "xb": xb_a,
                             "y0T": y0T[1]},
                            {"xb": x1_a, "acc": acc_a},
                        )
                    nc.sync.dma_start(d_lstm[ds(jv, CHUNK), :], acc_a[:])
                    x1_b = emit_x1chunk(1)     # chunk 2j+1 (odd parity)
                    acc_b = laccp.tile([CHUNK, H], f32, tag="lacc")
                    for u in range(CHUNK):
                        emit_pair(
                            u,
                            {"lhsT": l0_cols(y0T[0], y0T[1], u), "xb": xb_b,
                             "y0T": y0T[0]},
                            {"xb": x1_b, "acc": acc_b},
                        )
                    nc.sync.dma_start(d_lstm[ds(jv + CHUNK, CHUNK), :], acc_b[:])

                # ---- epilogue: L0 chunk 31 ; L1 chunks 30, 31 ----
                xb31 = emit_x0chunk(31 * CHUNK)
                last_h0 = None
                for u in range(CHUNK):
                    last_h0 = emit_pair(
                        u, {"lhsT": l0_cols(y0T[1], y0T[0], u), "xb": xb31,
                            "y0T": y0T[1]}, None)
                x1_30 = emit_x1chunk(0)
                acc30 = laccp.tile([CHUNK, H], f32, tag="lacc")
                for u in range(CHUNK):
                    emit_pair(u, None, {"xb": x1_30, "acc": acc30})
                nc.sync.dma_start(d_lstm[30 * CHUNK:31 * CHUNK, :], acc30[:])
                x1_31 = emit_x1chunk(1)
                acc31 = laccp.tile([CHUNK, H], f32, tag="lacc")
                last_h1 = None
                for u in range(CHUNK):
                    last_h1 = emit_pair(u, None, {"xb": x1_31, "acc": acc31})
                nc.sync.dma_start(d_lstm[31 * CHUNK:32 * CHUNK, :], acc31[:])

                # final state rows: h0, h1, c0, c1
                nc.sync.dma_start(d_state[0:1, :], last_h0[:])
                nc.sync.dma_start(d_state[1:2, :], last_h1[:])
                nc.sync.dma_start(d_state[2:3, :], cst0[:])
                nc.sync.dma_start(d_state[3:4, :], cst1[:])

            # ================= heads =================
            with tc.tile_pool(name="hd", bufs=1) as hd, \
                 tc.tile_pool(name="hdw", bufs=4) as hdw, \
                 tc.tile_pool(name="hsc", bufs=1) as hsc, \
                 tc.tile_pool(name="ph", bufs=4, space="PSUM") as ph, \
                 tc.tile_pool(name="ptr2", bufs=2, space="PSUM") as ptr2:

                iota = pers.tile([128, 32], f32)
                nc.sync.dma_start(iota[:], d_iota[:])

                logits = hd.tile([128, V2], f32, tag="logits")

                for mt in range(3):
                    # gather this tile's 128 routed lstm_out rows
                    ridx_sb = hsc.tile([128, 1], mybir.dt.int32, tag="ridx")
                    nc.sync.dma_start(ridx_sb[:], d_ridx[mt * 128:(mt + 1) * 128, :])
                    R = hsc.tile([128, H], f32, tag="R")
                    nc.gpsimd.indirect_dma_start(
                        out=R[:], out_offset=None, in_=d_lstm[:],
                        in_offset=bass.IndirectOffsetOnAxis(ap=ridx_sb[:, :1], axis=0),
                    )
                    # RT: [512, 128] as 4 [128,128] K-tiles
                    RT = hsc.tile([128, 4 * 128], f32, tag="RT")
                    for j in range(4):
                        ps = ptr2.tile([128, 128], f32, tag="tr2")
                        nc.tensor.transpose(ps[:], R[:, j * 128:(j + 1) * 128],
                                            ident[:])
                        nc.vector.tensor_copy(RT[:, j * 128:(j + 1) * 128], ps[:])

                    # logits = R @ Wc + bc
                    for n in range(V2 // 512):
                        ps = ph.tile([128, 512], f32, tag="ph")
                        for j in range(4):
                            w = hdw.tile([128, 512], f32, tag="hw")
                            nc.sync.dma_start(
                                w[:], d_wc[j * 128:(j + 1) * 128,
                                           n * 512:(n + 1) * 512])
                            nc.tensor.matmul(ps[:], lhsT=RT[:, j * 128:(j + 1) * 128],
                                             rhs=w[:], start=(j == 0), stop=False)
                        bv = hdw.tile([1, 512], f32, tag="hb")
                        nc.sync.dma_start(bv[:], d_bcv[:, n * 512:(n + 1) * 512])
                        nc.tensor.matmul(ps[:], lhsT=ones128[:], rhs=bv[:],
                                         start=False, stop=True)
                        nc.vector.tensor_copy(logits[:, n * 512:(n + 1) * 512], ps[:])

                    # row max via max8 on both halves
                    m8a = hsc.tile([128, 8], f32, tag="m8a")
                    m8b = hsc.tile([128, 8], f32, tag="m8b")
                    nc.vector.max(out=m8a[:], in_=logits[:, :HALF])
                    nc.vector.max(out=m8b[:], in_=logits[:, HALF:])
                    negmax = hsc.tile([128, 1], f32, tag="negmax")
                    nc.vector.tensor_tensor(negmax[:], m8a[:, 0:1], m8b[:, 0:1],
                                            op=OP.max)
                    nc.vector.tensor_scalar_mul(negmax[:], negmax[:], -1.0)

                    # logsumexp: exp(x - max) chunk-wise with accumulation
                    esums = hsc.tile([128, V2 // 512], f32, tag="esums")
                    escr = hsc.tile([128, 512], f32, tag="escr")
                    for n in range(V2 // 512):
                        nc.scalar.activation(escr[:], logits[:, n * 512:(n + 1) * 512],
                                             AF.Exp, bias=negmax[:],
                                             accum_out=esums[:, n:n + 1])
                    lse = hsc.tile([128, 1], f32, tag="lse")
                    nc.vector.tensor_reduce(lse[:], esums[:], axis=AX.X, op=OP.add)
                    nc.scalar.activation(lse[:], lse[:], AF.Ln)
                    nc.vector.tensor_scalar(lse[:], lse[:], negmax[:], None,
                                            op0=OP.subtract)

                    # nll = lse - (sum(R*wt) + bct)
                    wt_sb = hsc.tile([128, H], f32, tag="wt")
                    nc.sync.dma_start(wt_sb[:], d_wt[mt * 128:(mt + 1) * 128, :])
                    dot = hsc.tile([128, 1], f32, tag="dot")
                    dscr = hsc.tile([128, H], f32, tag="dscr")
                    nc.vector.scalar_tensor_tensor(
                        dscr[:], R[:], 1.0, wt_sb[:], op0=OP.mult, op1=OP.mult,
                        accum_out=dot[:])
                    bct_sb = hsc.tile([128, 1], f32, tag="bct")
                    nc.sync.dma_start(bct_sb[:], d_bct[mt * 128:(mt + 1) * 128, :])
                    nll = hsc.tile([128, 1], f32, tag="nll")
                    nc.vector.tensor_tensor(nll[:], dot[:], bct_sb[:], op=OP.add)
                    nc.vector.tensor_tensor(nll[:], lse[:], nll[:], op=OP.subtract)
                    nc.sync.dma_start(d_nll[mt * 128:(mt + 1) * 128, :], nll[:])

                    # ---- exact top-16 ----
                    # per half: top-8 (+idx), zap, next top-8 (+idx)
                    cand_v = hsc.tile([128, 32], f32, tag="cand_v")
                    cand_i = hsc.tile([128, 32], f32, tag="cand_i")
                    iu = hsc.tile([128, 8], mybir.dt.uint32, tag="iu")
                    for hix, (lo, hi) in enumerate([(0, HALF), (HALF, V2)]):
                        half = logits[:, lo:hi]
                        for r in range(2):
                            co = hix * 16 + r * 8
                            m8 = hsc.tile([128, 8], f32, tag="m8")
                            nc.vector.max(out=m8[:], in_=half)
                            nc.vector.max_index(out=iu[:], in_max=m8[:], in_values=half)
                            nc.vector.tensor_copy(cand_v[:, co:co + 8], m8[:])
                            nc.vector.tensor_copy(cand_i[:, co:co + 8], iu[:])
                            if hix == 1:
                                nc.vector.tensor_scalar_add(
                                    cand_i[:, co:co + 8], cand_i[:, co:co + 8],
                                    float(HALF))
                            if r == 0:
                                nc.vector.match_replace(out=half, in_to_replace=m8[:],
                                                        in_values=half,
                                                        imm_value=-1e30)
                    # top-16 of the 32 candidates
                    pred = hsc.tile([128, 16], f32, tag="pred")
                    pos = hsc.tile([128, 16], f32, tag="pos")
                    cv_copy = hsc.tile([128, 32], f32, tag="cvc")
                    nc.vector.tensor_copy(cv_copy[:], cand_v[:])
                    m16 = hsc.tile([128, 8], f32, tag="m16")
                    p8 = hsc.tile([128, 8], mybir.dt.uint32, tag="p8")
                    for r in range(2):
                        nc.vector.max(out=m16[:], in_=cv_copy[:])
                        nc.vector.max_index(out=p8[:], in_max=m16[:], in_values=cand_v[:])
                        nc.vector.tensor_copy(pos[:, r * 8:(r + 1) * 8], p8[:])
                        if r == 0:
                            nc.vector.match_replace(out=cv_copy[:], in_to_replace=m16[:],
                                                    in_values=cv_copy[:],
                                                    imm_value=-1e30)
                    # map candidate positions -> original vocab indices
                    eq = hsc.tile([128, 32], f32, tag="eq")
                    for kk in range(16):
                        nc.vector.tensor_scalar(eq[:], iota[:], pos[:, kk:kk + 1],
                                                None, op0=OP.is_equal)
                        nc.vector.scalar_tensor_tensor(
                            eq[:], eq[:], 1.0, cand_i[:], op0=OP.mult, op1=OP.mult,
                            accum_out=pred[:, kk:kk + 1])
                    nc.sync.dma_start(d_pred[mt * 128:(mt + 1) * 128, :], pred[:])

    _split_multi_waits(nc, bass_rust)
    return nc


def _get_program():
    if "nc" not in _CACHE:
        _CACHE["nc"] = _build_program()
    return _CACHE["nc"]


def kernel(pc, delta, clusters, target, h0, c0, pc_embed_w, delta_embed_w,
           W_ih0, W_hh0, b_ih0, b_hh0, W_ih1, W_hh1, b_ih1, b_hh1, Wc, bc):
    from concourse.bass_utils import run_bass_kernel_spmd

    pc = np.asarray(pc).astype(np.int64)
    delta = np.asarray(delta).astype(np.int64)
    clusters_np = np.asarray(clusters).astype(np.int64)
    target_np = np.asarray(target).astype(np.int64)
    f = lambda a: np.ascontiguousarray(np.asarray(a), dtype=np.float32)
    h0 = f(h0); c0 = f(c0)
    Wc = f(Wc); bc = f(bc)

    # host prep: embedding gather + transposes (pure data movement / sharding)
    x = np.concatenate([f(pc_embed_w)[pc], f(delta_embed_w)[delta]], axis=-1)
    xT = np.ascontiguousarray(x.T)                     # [512, 2048]
    wih0T = np.ascontiguousarray(f(W_ih0).T)           # [512, 2048]
    whh0T = np.ascontiguousarray(f(W_hh0).T)
    wih1T = np.ascontiguousarray(f(W_ih1).T)
    whh1T = np.ascontiguousarray(f(W_hh1).T)
    b0 = (f(b_ih0) + f(b_hh0)).reshape(1, G)
    b1 = (f(b_ih1) + f(b_hh1)).reshape(1, G)
    hcinit = np.stack([h0[0], h0[1], c0[0], c0[1]]).astype(np.float32)
    iota32 = np.tile(np.arange(32, dtype=np.float32), (128, 1))

    common = dict(xT=xT, wih0T=wih0T, whh0T=whh0T, wih1T=wih1T, whh1T=whh1T,
                  b0=b0, b1=b1, hcinit=hcinit, iota32=iota32)

    rows_by_c = [np.where(clusters_np == c)[0] for c in range(C)]
    counts = [len(r) for r in rows_by_c]
    assert max(counts) <= NCAP, f"cluster count {max(counts)} exceeds NCAP={NCAP}"

    in_maps = []
    for c in range(C):
        rows = rows_by_c[c]
        ridx = np.zeros(NCAP, dtype=np.int32)
        ridx[:len(rows)] = rows
        tr = target_np[ridx.astype(np.int64)]          # padded rows use row 0's
        WcT = np.ascontiguousarray(Wc[c].T)            # [20000, 512]
        wt = WcT[tr]                                   # [NCAP, 512]
        bct = bc[c][tr].reshape(NCAP, 1)
        wc_pad = np.zeros((H, V2), dtype=np.float32)
        wc_pad[:, :V] = Wc[c]
        bcv = np.full((1, V2), -1e30, dtype=np.float32)
        bcv[0, :V] = bc[c]
        m = dict(common)
        m.update(ridx=ridx.reshape(NCAP, 1), wt=np.ascontiguousarray(wt),
                 bct=np.ascontiguousarray(bct), wc=wc_pad, bcv=bcv)
        in_maps.append(m)

    nc = _get_program()
    import os
    trace = bool(int(os.environ.get("KERNEL_TRACE", "0")))
    res = run_bass_kernel_spmd(nc, in_maps, core_ids=list(range(C)), trace=trace)
    _CACHE["exec_time_ns"] = res.exec_time_ns
    _CACHE["results"] = res

    # ---- assemble outputs ----
    preds = np.zeros((B, 10), dtype=np.int32)
    loss = np.float32(0.0)
    for c in range(C):
        cnt = counts[c]
        if cnt == 0:
            continue
        rows = rows_by_c[c]
        nll = res.results[c]["nll_out"][:cnt, 0]
        loss = np.float32(loss + np.float32(nll.sum() / np.float32(cnt)))
        p = res.results[c]["pred_out"][:cnt, :10]
        preds[rows] = p.astype(np.int32)
    st = res.results[0]["state_out"]
    state = (np.stack([st[0], st[1]]).astype(np.float32),
             np.stack([st[2], st[3]]).astype(np.float32))
    return np.float32(loss), preds, state
